# revision 24
# baseline (speedup 1.0000x reference)
"""Multi-head attention (B=2, S=2048, D=1024, H=16) on 8 NeuronCores. v9.

Sharding: core c -> batch c//4, head-group c%4 (4 heads, 256 proj dims).

Evolved from the 251us v3 baseline via trace + microbench findings:
- ACT exp is the stream floor (0.833ns/elem free-size, engine-exclusive:
  walrus rejects InstActivation on DVE; DVE shift AluOps return 0 so no
  custom-op exp). Attacked by instruction width: a (2-unit + 1-unit) exp
  pattern per 3-unit period -- one [128,2048] activation over a 4-bank
  PSUM region + one [128,1024] over a 2-bank region. With the two 1-bank
  AV accumulators that is exactly 8 PSUM banks. 85 ACT instructions
  (124us busy) instead of 128 (142us).
- PE row-tile packing is real: 64-contraction score matmuls whose
  operands sit on opposite SBUF partition halves execute concurrently
  (129.5 vs 259ns measured). The period refill interleaves the two
  units' score matmuls (alternating h2 -> alternating partition halves).
  NOTE: accumulating (start=False) tiled matmuls may NOT overlap -- two
  concurrent halves accumulating PSUM fault the device even on distinct
  banks, so AV stays full-row (128-contraction, self-serializing).
- DMA: ~10us engine startup + ~1us SWDGE descriptor generation per
  dma_start (serial on the sync queue) -> 13 transfers total. x/w
  contraction chunks use the "(p c)" row permutation (chunk c = source
  rows {8p+c}), which cancels between x and W and makes each source run
  contiguous per partition (128 x 32KB descriptors per 4MB tensor).
  Issue order = consumption order: wk, xk, biases, wv, xv, wq, xq[cols
  0:512], xq[512:1024], wo, xq[1024:2048]. K/V projections chase the
  2MB chunk transfers; Q cols 0:512 project in the head (first stream
  block is qc4=0) and Q cols 512:2048 project as boundary lumps.
- Stream: 128 units u = (qc4 x4, j x2, ktp x8, h2 x2); per unit one
  [128, 2x512] score region slot pair (kt = 2ktp, 2ktp+1), exp'd per the
  big/small pattern, AV accumulated into per-h2 [65,512] ot tiles (ones
  column in the V stationary produces the softmax denominator row).
  Normalize per block on DVE + gpsimd partition_broadcast (gpsimd is
  ~4x slower than modeled for bulk copies and cannot touch PSUM).
- Lumps (Q col projections, out-projection of finished q-chunks) run at
  block boundaries on the briefly-free ot banks; a 5-period pt pool lets
  AV lag exp so lumps mostly displace AV work, not ACT. Tail projects
  the last 4 ytiles on all 8 freed banks in parallel.
- Idempotent partial score rewrites keep PE duty high for the HAM clock
  gate; warm-up/bridge fillers cover DMA waits in the head.
- bf16 inputs/weights (fp8 rejected: quantization blows the 2e-2 rel
  err budget). V bias folded into bo on the host (exact: attention rows
  sum to 1). Measured 237.3us (best), rel err 6.69e-3.
"""

import sys

sys.path.insert(0, "/opt/trn_rl_repo")

from contextlib import ExitStack

import numpy as np

import concourse.bacc as bacc
import concourse.mybir as mybir
import concourse.tile as tile
from concourse.bass_utils import run_bass_kernel_spmd

B = 2
S = 2048
D = 1024
H = 16
HD = 64
HPC = 4          # heads per core
DPC = HPC * HD   # 256 projection dims per core
NCORES = 8
SCALE = 8.0      # sqrt(HD)

F32 = mybir.dt.float32
BF16 = mybir.dt.bfloat16

DCH = D // 128   # 8 contraction chunks of 128
QT = S // 128    # 16 k-tiles of 128
NU = 128         # stream units: 4 qc4 x 2 j x 8 ktp x 2 h2

EXPF = mybir.ActivationFunctionType.Exp
IDENT = mybir.ActivationFunctionType.Identity


def unit(u):
    return (u >> 5), (u >> 4) & 1, (u >> 1) & 7, u & 1  # qc4, j, ktp, h2


def build_nc():
    nc = bacc.Bacc("TRN2", target_bir_lowering=False, debug=False, num_devices=NCORES)

    xq = nc.dram_tensor("xq_t", [D, S], BF16, kind="ExternalInput")
    xk = nc.dram_tensor("xk_t", [D, S], BF16, kind="ExternalInput")
    xv = nc.dram_tensor("xv_t", [D, S], BF16, kind="ExternalInput")
    wq = nc.dram_tensor("wq_t", [D, DPC], BF16, kind="ExternalInput")
    wk = nc.dram_tensor("wk_t", [D, DPC], BF16, kind="ExternalInput")
    wv = nc.dram_tensor("wv_t", [D, DPC], BF16, kind="ExternalInput")
    wo = nc.dram_tensor("wo_t", [DPC, D], BF16, kind="ExternalInput")
    bq = nc.dram_tensor("bq", [DPC, 1], F32, kind="ExternalInput")
    bk = nc.dram_tensor("bk", [DPC, 1], F32, kind="ExternalInput")
    y = nc.dram_tensor("y", [S, D], BF16, kind="ExternalOutput")

    with tile.TileContext(nc) as tc, ExitStack() as ctx:
        const = ctx.enter_context(tc.tile_pool(name="const", bufs=1))
        xkp = ctx.enter_context(tc.tile_pool(name="xkp", bufs=1))
        xvp = ctx.enter_context(tc.tile_pool(name="xvp", bufs=1))
        xqp = ctx.enter_context(tc.tile_pool(name="xqp", bufs=1))
        qkv = ctx.enter_context(tc.tile_pool(name="qkv", bufs=1))
        ptp = ctx.enter_context(tc.tile_pool(name="ptp", bufs=1))
        nrm = ctx.enter_context(tc.tile_pool(name="nrm", bufs=2))
        yp = ctx.enter_context(tc.tile_pool(name="yp", bufs=3))

        # ---- t=0: ACT exp-table preload + PE warm-up fodder ----
        dmy = const.tile([1, 16], F32, tag="dmy")
        nc.vector.memset(dmy[:], 0.0)
        dmy2 = const.tile([1, 16], F32, tag="dmy2")
        nc.scalar.activation(dmy2[:], dmy[:], EXPF)

        wrm32 = const.tile([128, 128], F32, tag="wrm32")
        nc.vector.memset(wrm32[:], 0.0)
        wrm = const.tile([128, 512], BF16, tag="wrm")
        nc.vector.tensor_copy(wrm[:, 0:128], wrm32[:])

        onesv32 = const.tile([128, HPC], F32, tag="onesv32")
        nc.vector.memset(onesv32[:], 1.0)

        # ---- weight / input SBUF tiles ----
        wqc = const.tile([128, DCH, DPC], BF16, tag="wqc")
        wkc = const.tile([128, DCH, DPC], BF16, tag="wkc")
        wvc = const.tile([128, DCH, DPC], BF16, tag="wvc")
        woc = const.tile([128, 2, D], BF16, tag="woc")
        bqc = const.tile([128, 2, 1], F32, tag="bqc")
        bkc = const.tile([128, 2, 1], F32, tag="bkc")
        wq_sb = [wqc[:, d, :] for d in range(DCH)]
        wk_sb = [wkc[:, d, :] for d in range(DCH)]
        wv_sb = [wvc[:, d, :] for d in range(DCH)]
        wo_sb = [woc[:, g, :] for g in range(2)]
        bq_sb = [bqc[:, hp, :] for hp in range(2)]
        bk_sb = [bkc[:, hp, :] for hp in range(2)]
        # x and w contraction chunks use the "(p c)" row permutation: chunk c
        # holds source rows {8p+c}. The same permutation on both operands
        # leaves every contraction sum unchanged, and makes each DMA source
        # run contiguous per partition (128 big descriptors per tensor).
        xk8 = xkp.tile([128, DCH, S], BF16, tag="xk8", name="xk8")
        xv8 = xvp.tile([128, DCH, S], BF16, tag="xv8", name="xv8")
        xq8 = {hf: xqp.tile([128, DCH, 1024], BF16, tag=f"xq8_{hf}", name=f"xq8_{hf}")
               for hf in range(2)}
        xk_sb = [xk8[:, d, :] for d in range(DCH)]
        xv_sb = [xv8[:, d, :] for d in range(DCH)]
        xq_sb = {(d, hf): xq8[hf][:, d, :] for hf in range(2) for d in range(DCH)}

        # ---- DMA issue order = consumption order (projections chase DMA):
        # biases, wk, xk*4, wq, wv, xv0, xq(0,0), xv1, xq(1,0), xv2, xv3,
        # wo, xq(*,1) ----
        # 12 transfers: SWDGE generation on the sync queue is ~1us each and
        # serializes, so transfer count trades against chase granularity.
        # Priority order = consumption order; biases ride between xk and wq.
        xk_ap = xk[:, :].rearrange("(p c) s -> p c s", c=DCH)
        xv_ap = xv[:, :].rearrange("(p c) s -> p c s", c=DCH)
        nc.sync.dma_start(wkc[:], wk[:, :].rearrange("(p c) m -> p c m", c=DCH))
        for t in range(2):
            nc.sync.dma_start(xk8[:, 4 * t:4 * t + 4, :], xk_ap[:, 4 * t:4 * t + 4, :])
        nc.sync.dma_start(bkc[:], bk[:, :].rearrange("(c p) o -> p c o", c=2))
        nc.sync.dma_start(bqc[:], bq[:, :].rearrange("(c p) o -> p c o", c=2))
        nc.sync.dma_start(wvc[:], wv[:, :].rearrange("(p c) m -> p c m", c=DCH))
        for t in range(2):
            nc.sync.dma_start(xv8[:, 4 * t:4 * t + 4, :], xv_ap[:, 4 * t:4 * t + 4, :])
        nc.sync.dma_start(wqc[:], wq[:, :].rearrange("(p c) m -> p c m", c=DCH))
        nc.sync.dma_start(
            xq8[0][:, :, 0:512], xq[:, 0:512].rearrange("(p c) s -> p c s", c=DCH))
        nc.sync.dma_start(
            xq8[0][:, :, 512:1024],
            xq[:, 512:1024].rearrange("(p c) s -> p c s", c=DCH))
        nc.sync.dma_start(woc[:], wo[:, :].rearrange("(c p) m -> p c m", c=2))
        nc.sync.dma_start(
            xq8[1][:], xq[:, 1024:2048].rearrange("(p c) s -> p c s", c=DCH))

        # ---- projection destinations ----
        kt_sb = [qkv.tile([128, S], BF16, tag=f"kt{j}", name=f"ktt{j}") for j in range(2)]
        qt_sb = [qkv.tile([128, S], BF16, tag=f"qt{j}", name=f"qtt{j}") for j in range(2)]
        v_sb = [qkv.tile([128, HPC * (HD + 1)], BF16, tag=f"v{st}", name=f"v{st}") for st in range(QT)]
        for st in range(QT):
            v4 = v_sb[st][:].rearrange("p (h w) -> p h w", h=HPC)
            nc.vector.tensor_copy(
                v4[:, :, HD:HD + 1],
                onesv32[:].rearrange("p (a b) -> p a b", b=1),
            )
        otn_sb = [qkv.tile([128, S], BF16, tag=f"otn{j}", name=f"otn{j}") for j in range(2)]

        def evac_add(idx, dst, src, bias_ap):
            # alternate PSUM->SBUF bias-add evacuation across DVE / ACT
            if idx % 2 == 0:
                nc.vector.tensor_scalar_add(dst, src, bias_ap)
            else:
                nc.scalar.activation(dst, src, IDENT, bias=bias_ap)

        # ================= phase 1: K, V, Q(cols 0:1024) =================
        with tc.tile_pool(name="ps_p", bufs=1, space="PSUM") as ps_p:
            # PE warm-up sized to ramp the HAM clock to 2.4GHz before the
            # first xk chunk lands (~11us)
            for i in range(28):
                wps = ps_p.tile([128, 512], F32, tag=f"pp{i % 8}", name=f"warm{i}")
                nc.tensor.matmul(wps[:], wrm[:, 0:128], wrm[:], start=True, stop=True)

            # K projection: weight-stationary, chases xk chunks
            kaccs = {}
            for hp in range(2):
                for pc in range(4):
                    kaccs[(hp, pc)] = ps_p.tile([128, 512], F32, tag=f"pp{hp * 4 + pc}",
                                                name=f"ppk{hp}{pc}")
            for d in range(DCH):
                for hp in range(2):
                    for pc in range(4):
                        nc.tensor.matmul(
                            kaccs[(hp, pc)][:],
                            wk_sb[d][:, hp * 128:(hp + 1) * 128],
                            xk_sb[d][:, pc * 512:(pc + 1) * 512],
                            start=(d == 0), stop=(d == DCH - 1),
                        )
            for hp in range(2):
                for pc in range(4):
                    evac_add(pc, kt_sb[hp][:, pc * 512:(pc + 1) * 512],
                             kaccs[(hp, pc)][:], bk_sb[hp])
            # bridge fillers on freed K banks: cover the gap until xq(0,0)
            # lands (PE duty for the HAM clock gate)
            for i in range(12):
                wps = ps_p.tile([128, 512], F32, tag=f"pp{4 + i % 4}",
                                name=f"kbridge{i}")
                nc.tensor.matmul(wps[:], wrm[:, 0:128], wrm[:],
                                 start=True, stop=True)

            # V projection: x-stationary, 2 rounds of 8 s-tiles; no bias
            # (folded into bo on host).
            for rnd in range(2):
                vaccs = {}
                for sti in range(8):
                    st = rnd * 8 + sti
                    vaccs[st] = ps_p.tile([128, DPC], F32, tag=f"pp{sti}",
                                          name=f"ppv{st}")
                for d in range(DCH):
                    for sti in range(8):
                        st = rnd * 8 + sti
                        nc.tensor.matmul(
                            vaccs[st][:],
                            xv_sb[d][:, st * 128:(st + 1) * 128],
                            wv_sb[d],
                            start=(d == 0), stop=(d == DCH - 1),
                        )
                for sti in range(8):
                    st = rnd * 8 + sti
                    v4 = v_sb[st][:].rearrange("p (h w) -> p h w", h=HPC)
                    if sti % 2 == 1:
                        nc.scalar.copy(
                            v4[:, :, 0:HD],
                            vaccs[st][:].rearrange("p (h w) -> p h w", h=HPC))
                    else:
                        nc.vector.tensor_copy(
                            v4[:, :, 0:HD],
                            vaccs[st][:].rearrange("p (h w) -> p h w", h=HPC))
            # Q projection cols 0:512 only (qc4 0) -- the stream's first
            # block; qc4 1..3 are projected as boundary lumps
            qaccs = {}
            for hp in range(2):
                qaccs[hp] = ps_p.tile([128, 512], F32, tag=f"pp{hp}",
                                      name=f"ppq{hp}")
            for d in range(DCH):
                for hp in range(2):
                    nc.tensor.matmul(
                        qaccs[hp][:],
                        wq_sb[d][:, hp * 128:(hp + 1) * 128],
                        xq_sb[(d, 0)][:, 0:512],
                        start=(d == 0), stop=(d == DCH - 1),
                    )
            for hp in range(2):
                # DVE-only: keep ACT clear for the first stream exps
                nc.vector.tensor_scalar_add(
                    qt_sb[hp][:, 0:512], qaccs[hp][:], bq_sb[hp])
            # bridge fillers on banks 4-7 while xv finishes streaming in
            for i in range(8):
                wps = ps_p.tile([128, 512], F32, tag=f"pp{4 + i % 4}",
                                name=f"qbridge{i}")
                nc.tensor.matmul(wps[:], wrm[:, 0:128], wrm[:],
                                 start=True, stop=True)


        # ================= phase 2: attention stream =================
        # Units u = (qc4, j, ktp, h2); periods of 3 units: exp-big(u,u+1)
        # from the 4-bank region, exp-small(u+2) from the 2-bank region.
        with tc.tile_pool(name="ps_m", bufs=1, space="PSUM") as ps_m:
            # PSUM: big region 4 banks + small region 2 banks + 2 AV accs
            WIDE_EXP = True

            def big_region(i):
                return ps_m.tile([128, 2048], F32, tag="rb", name=f"rb{i}")

            def small_region(i):
                return ps_m.tile([128, 1024], F32, tag="rs", name=f"rs{i}")

            def ot_tile(idx, name):
                return ps_m.tile([128, 512], F32, tag=f"ot{idx}", name=name)

            # pt pool: 3 big (2-unit) + 3 small (1-unit) tiles; AV may lag
            # exp by ~2 periods before the pool recycles
            ptb_t = [ptp.tile([128, 2048], BF16, tag=f"ptb{i}", name=f"ptb{i}")
                     for i in range(5)]
            pts_t = [ptp.tile([128, 1024], BF16, tag=f"pts{i}", name=f"pts{i}")
                     for i in range(5)]

            def emit_scores(u, reg, slot0, only=None):
                # scores for unit u into region slots [slot0, slot0+1]
                qc4, j, ktp, h2 = unit(u)
                r0 = h2 * 64
                for s in ((0, 1) if only is None else (only,)):
                    kt = 2 * ktp + s
                    c0 = (slot0 + s) * 512
                    nc.tensor.matmul(
                        reg[:, c0:c0 + 512],
                        kt_sb[j][r0:r0 + 64, kt * 128:(kt + 1) * 128],
                        qt_sb[j][r0:r0 + 64, qc4 * 512:(qc4 + 1) * 512],
                        start=True, stop=True,
                    )

            def emit_filler(u, reg, slot0, w=128):
                # idempotent rewrite (same operands as the real score matmul
                # prefix): PE-duty filler for the HAM clock gate
                qc4, j, ktp, h2 = unit(u)
                r0 = h2 * 64
                kt = 2 * ktp
                nc.tensor.matmul(
                    reg[:, slot0 * 512:slot0 * 512 + w],
                    kt_sb[j][r0:r0 + 64, kt * 128:(kt + 1) * 128],
                    qt_sb[j][r0:r0 + 64, qc4 * 512:qc4 * 512 + w],
                    start=True, stop=True,
                )

            def emit_av(u, ot, ptt, sb, first, last):
                # full-row AV (used for the small slot and at boundaries)
                qc4, j, ktp, h2 = unit(u)
                h = 2 * j + h2
                for s in range(2):
                    kt = 2 * ktp + s
                    c0 = (sb + s) * 512
                    nc.tensor.matmul(
                        ot[0:65, :],
                        v_sb[kt][:, h * 65:h * 65 + 65],
                        ptt[:, c0:c0 + 512],
                        start=(first and s == 0),
                        stop=(last and s == 1),
                    )

            def emit_av_pair(uA, uB, ptt):
                # split both units' AV into 64-row halves, interleaved so
                # adjacent matmuls alternate BOTH partition half and PSUM
                # bank (otA vs otB) -- concurrent same-bank accumulation
                # faults the PE (found the hard way)
                _, jA, ktpA, h2A = unit(uA)
                _, jB, ktpB, h2B = unit(uB)
                otA, otB = state["ot"][h2A], state["ot"][h2B]
                hA, hB = 2 * jA + h2A, 2 * jB + h2B
                for s in range(2):
                    ktA, ktB = 2 * ktpA + s, 2 * ktpB + s
                    for step in range(2):
                        hhA, hhB = step, 1 - step
                        cA, cB = s * 512, (2 + s) * 512
                        nc.tensor.matmul(
                            otA[0:65, :],
                            v_sb[ktA][hhA * 64:(hhA + 1) * 64, hA * 65:hA * 65 + 65],
                            ptt[hhA * 64:(hhA + 1) * 64, cA:cA + 512],
                            start=(ktpA == 0 and s == 0 and step == 0),
                            stop=(ktpA == 7 and s == 1 and step == 1),
                        )
                        nc.tensor.matmul(
                            otB[0:65, :],
                            v_sb[ktB][hhB * 64:(hhB + 1) * 64, hB * 65:hB * 65 + 65],
                            ptt[hhB * 64:(hhB + 1) * 64, cB:cB + 512],
                            start=(ktpB == 0 and s == 0 and step == 0),
                            stop=(ktpB == 7 and s == 1 and step == 1),
                        )

            def emit_norm(qc4, j, h2, ot):
                # otn[d, q] = ot[d, q] / ot[64, q] for this (block, head)
                drow = nrm.tile([1, 512], F32, tag="drow", name=f"dr{qc4}{j}{h2}")
                nc.vector.tensor_copy(drow[:], ot[64:65, :])
                otr = nrm.tile([HD, 512], BF16, tag="otr", name=f"otr{qc4}{j}{h2}")
                nc.vector.tensor_copy(otr[:], ot[0:HD, :])
                r32 = nrm.tile([1, 512], F32, tag="r32", name=f"r32{qc4}{j}{h2}")
                nc.vector.reciprocal_approx_fast(r32[:], drow[:])
                rb = nrm.tile([1, 512], BF16, tag="rb", name=f"rbn{qc4}{j}{h2}")
                nc.vector.tensor_copy(rb[:], r32[:])
                sc = nrm.tile([HD, 512], BF16, tag="sc", name=f"sc{qc4}{j}{h2}")
                nc.gpsimd.partition_broadcast(sc[:], rb[:])
                nc.vector.tensor_mul(
                    otn_sb[j][h2 * 64:h2 * 64 + 64,
                              qc4 * 512:qc4 * 512 + 512],
                    otr[:], sc[:])

            def emit_qp2(hp, pc):
                # Q projection for cols 1024:2048 (qc4 2,3): one [128,512]
                # column block on a boundary-window bank; evac on DVE only
                # (ACT is the stream bottleneck)
                acc = ot_tile(hp % 2, f"qp2_{hp}{pc}")
                for d in range(DCH):
                    if pc == 1:
                        xsrc = xq_sb[(d, 0)][:, 512:1024]
                    else:
                        xsrc = xq_sb[(d, 1)][:, (pc - 2) * 512:(pc - 1) * 512]
                    nc.tensor.matmul(
                        acc[:],
                        wq_sb[d][:, hp * 128:(hp + 1) * 128],
                        xsrc,
                        start=(d == 0), stop=(d == DCH - 1),
                    )
                nc.vector.tensor_scalar_add(
                    qt_sb[hp][:, pc * 512:(pc + 1) * 512], acc[:], bq_sb[hp])

            def emit_ytile(yt, tail=False):
                ysb = yp.tile([128, D], BF16, tag="y", name=f"ysb{yt}")
                for dc in range(2):
                    op = ot_tile(dc, f"op{yt}{dc}")
                    for g in range(2):
                        nc.tensor.matmul(
                            op[:],
                            otn_sb[g][:, yt * 128:(yt + 1) * 128],
                            wo_sb[g][:, dc * 512:(dc + 1) * 512],
                            start=(g == 0), stop=(g == 1),
                        )
                    if tail and dc == 1:
                        nc.scalar.copy(ysb[:, dc * 512:(dc + 1) * 512], op[:])
                    else:
                        nc.vector.tensor_copy(ysb[:, dc * 512:(dc + 1) * 512], op[:])
                    if tail:
                        nc.sync.dma_start(
                            y[yt * 128:(yt + 1) * 128, dc * 512:(dc + 1) * 512],
                            ysb[:, dc * 512:(dc + 1) * 512])
                if not tail:
                    nc.sync.dma_start(y[yt * 128:(yt + 1) * 128, :], ysb[:])

            # lump schedule per boundary index (after block b ends)
            #   b = 1..7 at units 16,32,...,112
            lumps = {
                1: [("qp2", 0, 1), ("qp2", 1, 1)],
                2: [("qp2", 0, 2), ("qp2", 1, 2)],
                3: [("y", 0), ("y", 1), ("y", 2), ("y", 3)],
                4: [("qp2", 0, 3), ("qp2", 1, 3)],
                5: [("y", 4), ("y", 5), ("y", 6), ("y", 7)],
                6: [("y", 8), ("y", 9), ("y", 10), ("y", 11)],
            }

            # ---- prologue: scores(0), scores(1) -> big0; scores(2) -> small0
            big = big_region(0)
            small = small_region(0)
            emit_scores(0, big, 0)
            emit_scores(1, big, 2)
            emit_scores(2, small, 0)

            state = {"ot": [ot_tile(0, "otA0"), ot_tile(1, "otB0")], "blk": 0}

            def block_end(qc4, j):
                emit_norm(qc4, j, 0, state["ot"][0])
                emit_norm(qc4, j, 1, state["ot"][1])
                state["blk"] += 1
                blk = state["blk"]
                for lump in lumps.get(blk, []):
                    if lump[0] == "qp2":
                        emit_qp2(lump[1], lump[2])
                    else:
                        emit_ytile(lump[1])
                state["ot"] = [ot_tile(0, f"otA{blk}"), ot_tile(1, f"otB{blk}")]

            def do_av(u, ptt, sb):
                qc4, j, ktp, h2 = unit(u)
                emit_av(u, state["ot"][h2], ptt, sb,
                        first=(ktp == 0), last=(ktp == 7))
                if ktp == 7 and h2 == 1:
                    block_end(qc4, j)

            for p in range(43):  # 43 periods x 3 units cover 128 units
                u0 = 3 * p
                ptb = ptb_t[p % 5]
                pts = pts_t[p % 5]
                # exp-big over units u0, u0+1
                if WIDE_EXP:
                    nc.scalar.activation(ptb[:], big[:], EXPF, scale=1.0 / SCALE)
                else:
                    nc.scalar.activation(ptb[:, 0:1024], big[:, 0:1024], EXPF,
                                         scale=1.0 / SCALE)
                    nc.scalar.activation(ptb[:, 1024:2048], big[:, 1024:2048],
                                         EXPF, scale=1.0 / SCALE)
                # refill big with scores(u0+3), (u0+4) in the exp shadow
                nxt_big = big_region(p + 1)
                if u0 + 4 < NU:
                    # interleave the two units' score matmuls: adjacent
                    # matmuls alternate partition halves -> PE row-tile
                    # packing (2x throughput on 64-contraction matmuls)
                    emit_scores(u0 + 3, nxt_big, 0, only=0)
                    emit_scores(u0 + 4, nxt_big, 2, only=0)
                    emit_scores(u0 + 3, nxt_big, 0, only=1)
                    emit_scores(u0 + 4, nxt_big, 2, only=1)
                    emit_filler(u0 + 3, nxt_big, 0)
                    emit_filler(u0 + 4, nxt_big, 2)
                elif u0 + 3 < NU:
                    emit_scores(u0 + 3, nxt_big, 0)
                    emit_filler(u0 + 3, nxt_big, 0)
                if True or (u0 >> 4) != ((u0 + 1) >> 4):  # PAIR_AV disabled
                    # pair spans a block boundary: full-row AVs around the
                    # block end (the split halves would straddle the ot swap)
                    do_av(u0, ptb, 0)
                    do_av(u0 + 1, ptb, 2)
                else:
                    emit_av_pair(u0, u0 + 1, ptb)
                    if (u0 + 1) & 15 == 15:
                        block_end(*unit(u0 + 1)[0:2])

                # exp-small over unit u0+2
                u2 = u0 + 2
                if u2 < NU:
                    nc.scalar.activation(pts[:], small[:], EXPF,
                                         scale=1.0 / SCALE)
                nxt_small = small_region(p + 1)
                if u2 + 3 < NU:
                    emit_scores(u2 + 3, nxt_small, 0)
                    emit_filler(u2 + 3, nxt_small, 0)
                if u2 < NU:
                    do_av(u2, pts, 0)

                big = nxt_big
                small = nxt_small

            # ---- tail: out-projection of q rows 1536:2048 on parallel
            # banks (ring + ot tags are all free now) ----
            for i in range(4):
                svd = ps_m.tile([128, 1024], F32, tag="rs", name=f"warmt{i}")
                nc.tensor.matmul(svd[:, 0:512], wrm[:, 0:128], wrm[:],
                                 start=True, stop=True)
            tob = ps_m.tile([128, 2048], F32, tag="rb", name="tailb")
            tos = ps_m.tile([128, 1024], F32, tag="rs", name="tails")
            taccs = [tob[:, 0:512], tob[:, 512:1024], tob[:, 1024:1536],
                     tob[:, 1536:2048], tos[:, 0:512], tos[:, 512:1024],
                     ot_tile(0, "tailo0")[:], ot_tile(1, "tailo1")[:]]
            for i, yt in enumerate(range(12, 16)):
                ysb = yp.tile([128, D], BF16, tag="y", name=f"ysb{yt}")
                for dc in range(2):
                    op = taccs[i * 2 + dc]
                    for g in range(2):
                        nc.tensor.matmul(
                            op,
                            otn_sb[g][:, yt * 128:(yt + 1) * 128],
                            wo_sb[g][:, dc * 512:(dc + 1) * 512],
                            start=(g == 0), stop=(g == 1),
                        )
                    if dc == 1:
                        nc.scalar.copy(ysb[:, dc * 512:(dc + 1) * 512], op)
                    else:
                        nc.vector.tensor_copy(ysb[:, dc * 512:(dc + 1) * 512], op)
                    nc.sync.dma_start(
                        y[yt * 128:(yt + 1) * 128, dc * 512:(dc + 1) * 512],
                        ysb[:, dc * 512:(dc + 1) * 512])

    nc.compile()
    return nc


_NC_CACHE = None


def _get_nc():
    global _NC_CACHE
    if _NC_CACHE is None:
        _NC_CACHE = build_nc()
    return _NC_CACHE


def shard_inputs(query, key, value, Wq, bq, Wk, bk, Wv, bv, Wo, bo):
    """Build the 8 per-core input maps (host-side shard + transpose)."""
    import ml_dtypes
    f = np.float32
    bf = ml_dtypes.bfloat16
    in_maps = []
    for c in range(NCORES):
        b = c // 4
        g = c % 4
        hs = slice(g * DPC, (g + 1) * DPC)
        in_maps.append({
            "xq_t": np.ascontiguousarray(np.asarray(query[b], f).T.astype(bf)),
            "xk_t": np.ascontiguousarray(np.asarray(key[b], f).T.astype(bf)),
            "xv_t": np.ascontiguousarray(np.asarray(value[b], f).T.astype(bf)),
            "wq_t": np.ascontiguousarray(np.asarray(Wq[hs, :], f).T.astype(bf)),
            "wk_t": np.ascontiguousarray(np.asarray(Wk[hs, :], f).T.astype(bf)),
            "wv_t": np.ascontiguousarray(np.asarray(Wv[hs, :], f).T.astype(bf)),
            "wo_t": np.ascontiguousarray(np.asarray(Wo[:, hs], f).T.astype(bf)),
            "bq": np.asarray(bq[hs], f).reshape(DPC, 1).copy(),
            "bk": np.asarray(bk[hs], f).reshape(DPC, 1).copy(),
        })
    return in_maps


def kernel(query, key, value, Wq, bq, Wk, bk, Wv, bv, Wo, bo, **run_kwargs):
    nc = _get_nc()
    in_maps = shard_inputs(query, key, value, Wq, bq, Wk, bk, Wv, bv, Wo, bo)
    res = run_bass_kernel_spmd(nc, in_maps, core_ids=list(range(NCORES)),
                               **run_kwargs)
    out = np.zeros((B, S, D), np.float32)
    for c in range(NCORES):
        out[c // 4] += np.asarray(res.results[c]["y"], np.float32)
    # V bias folded here: attention rows sum to 1, so +bv passes through
    # attention unchanged and contributes bv @ Wo.T to every output row.
    bo_eff = np.asarray(bo, np.float32) + np.asarray(bv, np.float32) @ np.asarray(Wo, np.float32).T
    out += bo_eff
    if run_kwargs:
        kernel.last_result = res
    return out


# revision 25
# speedup vs baseline: 1.0184x; 1.0184x over previous
"""Multi-head attention (B=2, S=2048, D=1024, H=16) on 8 NeuronCores. v9.

Sharding: core c -> batch c//4, head-group c%4 (4 heads, 256 proj dims).

Evolved from the 251us v3 baseline via trace + microbench findings:
- ACT exp is the stream floor (0.833ns/elem free-size, engine-exclusive:
  walrus rejects InstActivation on DVE; DVE shift AluOps return 0 so no
  custom-op exp). Attacked by instruction width: a (2-unit + 1-unit) exp
  pattern per 3-unit period -- one [128,2048] activation over a 4-bank
  PSUM region + one [128,1024] over a 2-bank region. With the two 1-bank
  AV accumulators that is exactly 8 PSUM banks. 85 ACT instructions
  (124us busy) instead of 128 (142us).
- PE row-tile packing is real: 64-contraction score matmuls whose
  operands sit on opposite SBUF partition halves execute concurrently
  (129.5 vs 259ns measured). The period refill interleaves the two
  units' score matmuls (alternating h2 -> alternating partition halves).
  NOTE: accumulating (start=False) tiled matmuls may NOT overlap -- two
  concurrent halves accumulating PSUM fault the device even on distinct
  banks, so AV stays full-row (128-contraction, self-serializing).
- DMA: ~10us engine startup + ~1us SWDGE descriptor generation per
  dma_start (serial on the sync queue) -> 13 transfers total. x/w
  contraction chunks use the "(p c)" row permutation (chunk c = source
  rows {8p+c}), which cancels between x and W and makes each source run
  contiguous per partition (128 x 32KB descriptors per 4MB tensor).
  Issue order = consumption order: wk, xk, biases, wv, xv, wq, xq[cols
  0:512], xq[512:1024], wo, xq[1024:2048]. K/V projections chase the
  2MB chunk transfers; Q cols 0:512 project in the head (first stream
  block is qc4=0) and Q cols 512:2048 project as boundary lumps.
- Stream: 128 units u = (qc4 x4, j x2, ktp x8, h2 x2); per unit one
  [128, 2x512] score region slot pair (kt = 2ktp, 2ktp+1), exp'd per the
  big/small pattern, AV accumulated into per-h2 [65,512] ot tiles (ones
  column in the V stationary produces the softmax denominator row).
  Normalize per block on DVE + gpsimd partition_broadcast (gpsimd is
  ~4x slower than modeled for bulk copies and cannot touch PSUM).
- Lumps (Q col projections, out-projection of finished q-chunks) run at
  block boundaries on the briefly-free ot banks; a 5-period pt pool lets
  AV lag exp so lumps mostly displace AV work, not ACT. Tail projects
  the last 4 ytiles on all 8 freed banks in parallel.
- Idempotent partial score rewrites keep PE duty high for the HAM clock
  gate; warm-up/bridge fillers cover DMA waits in the head.
- bf16 inputs/weights (fp8 rejected: quantization blows the 2e-2 rel
  err budget). V bias folded into bo on the host (exact: attention rows
  sum to 1). Measured 237.3us (best), rel err 6.69e-3.
"""

import sys

sys.path.insert(0, "/opt/trn_rl_repo")

from contextlib import ExitStack

import numpy as np

import concourse.bacc as bacc
import concourse.mybir as mybir
import concourse.tile as tile
from concourse.bass_utils import run_bass_kernel_spmd

B = 2
S = 2048
D = 1024
H = 16
HD = 64
HPC = 4          # heads per core
DPC = HPC * HD   # 256 projection dims per core
NCORES = 8
SCALE = 8.0      # sqrt(HD)

F32 = mybir.dt.float32
BF16 = mybir.dt.bfloat16

DCH = D // 128   # 8 contraction chunks of 128
QT = S // 128    # 16 k-tiles of 128
NU = 128         # stream units: 4 qc4 x 2 j x 8 ktp x 2 h2

EXPF = mybir.ActivationFunctionType.Exp
IDENT = mybir.ActivationFunctionType.Identity


def unit(u):
    return (u >> 5), (u >> 4) & 1, (u >> 1) & 7, u & 1  # qc4, j, ktp, h2


def build_nc():
    nc = bacc.Bacc("TRN2", target_bir_lowering=False, debug=False, num_devices=NCORES)

    xq = nc.dram_tensor("xq_t", [D, S], BF16, kind="ExternalInput")
    xk = nc.dram_tensor("xk_t", [D, S], BF16, kind="ExternalInput")
    xv = nc.dram_tensor("xv_t", [D, S], BF16, kind="ExternalInput")
    wq = nc.dram_tensor("wq_t", [D, DPC], BF16, kind="ExternalInput")
    wk = nc.dram_tensor("wk_t", [D, DPC], BF16, kind="ExternalInput")
    wv = nc.dram_tensor("wv_t", [D, DPC], BF16, kind="ExternalInput")
    wo = nc.dram_tensor("wo_t", [DPC, D], BF16, kind="ExternalInput")
    bq = nc.dram_tensor("bq", [DPC, 1], F32, kind="ExternalInput")
    bk = nc.dram_tensor("bk", [DPC, 1], F32, kind="ExternalInput")
    y = nc.dram_tensor("y", [S, D], BF16, kind="ExternalOutput")

    with tile.TileContext(nc) as tc, ExitStack() as ctx:
        const = ctx.enter_context(tc.tile_pool(name="const", bufs=1))
        xkp = ctx.enter_context(tc.tile_pool(name="xkp", bufs=1))
        xvp = ctx.enter_context(tc.tile_pool(name="xvp", bufs=1))
        xqp = ctx.enter_context(tc.tile_pool(name="xqp", bufs=1))
        qkv = ctx.enter_context(tc.tile_pool(name="qkv", bufs=1))
        ptp = ctx.enter_context(tc.tile_pool(name="ptp", bufs=1))
        nrm = ctx.enter_context(tc.tile_pool(name="nrm", bufs=2))
        yp = ctx.enter_context(tc.tile_pool(name="yp", bufs=3))

        # ---- t=0: ACT exp-table preload + PE warm-up fodder ----
        dmy = const.tile([1, 16], F32, tag="dmy")
        nc.vector.memset(dmy[:], 0.0)
        dmy2 = const.tile([1, 16], F32, tag="dmy2")
        nc.scalar.activation(dmy2[:], dmy[:], EXPF)

        wrm32 = const.tile([128, 128], F32, tag="wrm32")
        nc.vector.memset(wrm32[:], 0.0)
        wrm = const.tile([128, 512], BF16, tag="wrm")
        nc.vector.tensor_copy(wrm[:, 0:128], wrm32[:])

        onesv32 = const.tile([128, HPC], F32, tag="onesv32")
        nc.vector.memset(onesv32[:], 1.0)

        # ---- weight / input SBUF tiles ----
        wqc = const.tile([128, DCH, DPC], BF16, tag="wqc")
        wkc = const.tile([128, DCH, DPC], BF16, tag="wkc")
        wvc = const.tile([128, DCH, DPC], BF16, tag="wvc")
        woc = const.tile([128, 2, D], BF16, tag="woc")
        bqc = const.tile([128, 2, 1], F32, tag="bqc")
        bkc = const.tile([128, 2, 1], F32, tag="bkc")
        wq_sb = [wqc[:, d, :] for d in range(DCH)]
        wk_sb = [wkc[:, d, :] for d in range(DCH)]
        wv_sb = [wvc[:, d, :] for d in range(DCH)]
        wo_sb = [woc[:, g, :] for g in range(2)]
        bq_sb = [bqc[:, hp, :] for hp in range(2)]
        bk_sb = [bkc[:, hp, :] for hp in range(2)]
        # x and w contraction chunks use the "(p c)" row permutation: chunk c
        # holds source rows {8p+c}. The same permutation on both operands
        # leaves every contraction sum unchanged, and makes each DMA source
        # run contiguous per partition (128 big descriptors per tensor).
        xk8 = xkp.tile([128, DCH, S], BF16, tag="xk8", name="xk8")
        xv8 = xvp.tile([128, DCH, S], BF16, tag="xv8", name="xv8")
        xq8 = {hf: xqp.tile([128, DCH, 1024], BF16, tag=f"xq8_{hf}", name=f"xq8_{hf}")
               for hf in range(2)}
        xk_sb = [xk8[:, d, :] for d in range(DCH)]
        xv_sb = [xv8[:, d, :] for d in range(DCH)]
        xq_sb = {(d, hf): xq8[hf][:, d, :] for hf in range(2) for d in range(DCH)}

        # ---- DMA issue order = consumption order (projections chase DMA):
        # biases, wk, xk*4, wq, wv, xv0, xq(0,0), xv1, xq(1,0), xv2, xv3,
        # wo, xq(*,1) ----
        # 12 transfers: SWDGE generation on the sync queue is ~1us each and
        # serializes, so transfer count trades against chase granularity.
        # Priority order = consumption order; biases ride between xk and wq.
        xk_ap = xk[:, :].rearrange("(p c) s -> p c s", c=DCH)
        xv_ap = xv[:, :].rearrange("(p c) s -> p c s", c=DCH)
        nc.sync.dma_start(wkc[:], wk[:, :].rearrange("(p c) m -> p c m", c=DCH))
        for t in range(2):
            nc.sync.dma_start(xk8[:, 4 * t:4 * t + 4, :], xk_ap[:, 4 * t:4 * t + 4, :])
        nc.sync.dma_start(bkc[:], bk[:, :].rearrange("(c p) o -> p c o", c=2))
        nc.sync.dma_start(bqc[:], bq[:, :].rearrange("(c p) o -> p c o", c=2))
        nc.sync.dma_start(wvc[:], wv[:, :].rearrange("(p c) m -> p c m", c=DCH))
        for t in range(2):
            nc.sync.dma_start(xv8[:, 4 * t:4 * t + 4, :], xv_ap[:, 4 * t:4 * t + 4, :])
        nc.sync.dma_start(wqc[:], wq[:, :].rearrange("(p c) m -> p c m", c=DCH))
        nc.sync.dma_start(
            xq8[0][:, :, 0:512], xq[:, 0:512].rearrange("(p c) s -> p c s", c=DCH))
        nc.sync.dma_start(
            xq8[0][:, :, 512:1024],
            xq[:, 512:1024].rearrange("(p c) s -> p c s", c=DCH))
        nc.sync.dma_start(woc[:], wo[:, :].rearrange("(c p) m -> p c m", c=2))
        nc.sync.dma_start(
            xq8[1][:], xq[:, 1024:2048].rearrange("(p c) s -> p c s", c=DCH))

        # ---- projection destinations ----
        kt_sb = [qkv.tile([128, S], BF16, tag=f"kt{j}", name=f"ktt{j}") for j in range(2)]
        qt_sb = [qkv.tile([128, S], BF16, tag=f"qt{j}", name=f"qtt{j}") for j in range(2)]
        v_sb = [qkv.tile([128, HPC * (HD + 1)], BF16, tag=f"v{st}", name=f"v{st}") for st in range(QT)]
        for st in range(QT):
            v4 = v_sb[st][:].rearrange("p (h w) -> p h w", h=HPC)
            nc.vector.tensor_copy(
                v4[:, :, HD:HD + 1],
                onesv32[:].rearrange("p (a b) -> p a b", b=1),
            )
        otn_sb = [qkv.tile([128, S], BF16, tag=f"otn{j}", name=f"otn{j}") for j in range(2)]

        def evac_add(idx, dst, src, bias_ap):
            # alternate PSUM->SBUF bias-add evacuation across DVE / ACT
            if idx % 2 == 0:
                nc.vector.tensor_scalar_add(dst, src, bias_ap)
            else:
                nc.scalar.activation(dst, src, IDENT, bias=bias_ap)

        # ================= phase 1: K, V, Q(cols 0:1024) =================
        with tc.tile_pool(name="ps_p", bufs=1, space="PSUM") as ps_p:
            # PE warm-up sized to ramp the HAM clock to 2.4GHz before the
            # first xk chunk lands (~11us)
            for i in range(28):
                wps = ps_p.tile([128, 512], F32, tag=f"pp{i % 8}", name=f"warm{i}")
                nc.tensor.matmul(wps[:], wrm[:, 0:128], wrm[:], start=True, stop=True)

            # K projection: weight-stationary, chases xk chunks
            kaccs = {}
            for hp in range(2):
                for pc in range(4):
                    kaccs[(hp, pc)] = ps_p.tile([128, 512], F32, tag=f"pp{hp * 4 + pc}",
                                                name=f"ppk{hp}{pc}")
            for d in range(DCH):
                for hp in range(2):
                    for pc in range(4):
                        nc.tensor.matmul(
                            kaccs[(hp, pc)][:],
                            wk_sb[d][:, hp * 128:(hp + 1) * 128],
                            xk_sb[d][:, pc * 512:(pc + 1) * 512],
                            start=(d == 0), stop=(d == DCH - 1),
                        )
            for hp in range(2):
                for pc in range(4):
                    evac_add(pc, kt_sb[hp][:, pc * 512:(pc + 1) * 512],
                             kaccs[(hp, pc)][:], bk_sb[hp])
            # bridge fillers on freed K banks: cover the gap until xq(0,0)
            # lands (PE duty for the HAM clock gate)
            for i in range(12):
                wps = ps_p.tile([128, 512], F32, tag=f"pp{4 + i % 4}",
                                name=f"kbridge{i}")
                nc.tensor.matmul(wps[:], wrm[:, 0:128], wrm[:],
                                 start=True, stop=True)

            # V projection: x-stationary, 2 rounds of 8 s-tiles; no bias
            # (folded into bo on host).
            for rnd in range(2):
                vaccs = {}
                for sti in range(8):
                    st = rnd * 8 + sti
                    vaccs[st] = ps_p.tile([128, DPC], F32, tag=f"pp{sti}",
                                          name=f"ppv{st}")
                for d in range(DCH):
                    for sti in range(8):
                        st = rnd * 8 + sti
                        nc.tensor.matmul(
                            vaccs[st][:],
                            xv_sb[d][:, st * 128:(st + 1) * 128],
                            wv_sb[d],
                            start=(d == 0), stop=(d == DCH - 1),
                        )
                for sti in range(8):
                    st = rnd * 8 + sti
                    v4 = v_sb[st][:].rearrange("p (h w) -> p h w", h=HPC)
                    if sti % 2 == 1:
                        nc.scalar.copy(
                            v4[:, :, 0:HD],
                            vaccs[st][:].rearrange("p (h w) -> p h w", h=HPC))
                    else:
                        nc.vector.tensor_copy(
                            v4[:, :, 0:HD],
                            vaccs[st][:].rearrange("p (h w) -> p h w", h=HPC))
            # Q projection cols 0:512 only (qc4 0) -- the stream's first
            # block; qc4 1..3 are projected as boundary lumps
            qaccs = {}
            for hp in range(2):
                qaccs[hp] = ps_p.tile([128, 512], F32, tag=f"pp{hp}",
                                      name=f"ppq{hp}")
            for d in range(DCH):
                for hp in range(2):
                    nc.tensor.matmul(
                        qaccs[hp][:],
                        wq_sb[d][:, hp * 128:(hp + 1) * 128],
                        xq_sb[(d, 0)][:, 0:512],
                        start=(d == 0), stop=(d == DCH - 1),
                    )
            for hp in range(2):
                # DVE-only: keep ACT clear for the first stream exps
                nc.vector.tensor_scalar_add(
                    qt_sb[hp][:, 0:512], qaccs[hp][:], bq_sb[hp])
            # bridge fillers on banks 4-7 while xv finishes streaming in
            for i in range(8):
                wps = ps_p.tile([128, 512], F32, tag=f"pp{4 + i % 4}",
                                name=f"qbridge{i}")
                nc.tensor.matmul(wps[:], wrm[:, 0:128], wrm[:],
                                 start=True, stop=True)


        # ================= phase 2: attention stream =================
        # Units u = (qc4, j, ktp, h2); periods of 3 units: exp-big(u,u+1)
        # from the 4-bank region, exp-small(u+2) from the 2-bank region.
        with tc.tile_pool(name="ps_m", bufs=1, space="PSUM") as ps_m:
            # PSUM: big region 4 banks + small region 2 banks + 2 AV accs
            WIDE_EXP = True

            def big_region(i):
                return ps_m.tile([128, 2048], F32, tag="rb", name=f"rb{i}")

            def small_region(i):
                return ps_m.tile([128, 1024], F32, tag="rs", name=f"rs{i}")

            def ot_tile(idx, name):
                return ps_m.tile([128, 512], F32, tag=f"ot{idx}", name=name)

            # pt pool: 3 big (2-unit) + 3 small (1-unit) tiles; AV may lag
            # exp by ~2 periods before the pool recycles
            ptb_t = [ptp.tile([128, 2048], BF16, tag=f"ptb{i}", name=f"ptb{i}")
                     for i in range(5)]
            pts_t = [ptp.tile([128, 1024], BF16, tag=f"pts{i}", name=f"pts{i}")
                     for i in range(5)]

            def emit_scores(u, reg, slot0, only=None):
                # scores for unit u into region slots [slot0, slot0+1]
                qc4, j, ktp, h2 = unit(u)
                r0 = h2 * 64
                for s in ((0, 1) if only is None else (only,)):
                    kt = 2 * ktp + s
                    c0 = (slot0 + s) * 512
                    nc.tensor.matmul(
                        reg[:, c0:c0 + 512],
                        kt_sb[j][r0:r0 + 64, kt * 128:(kt + 1) * 128],
                        qt_sb[j][r0:r0 + 64, qc4 * 512:(qc4 + 1) * 512],
                        start=True, stop=True,
                    )

            def emit_filler(u, reg, slot0, w=128):
                # idempotent rewrite (same operands as the real score matmul
                # prefix): PE-duty filler for the HAM clock gate
                qc4, j, ktp, h2 = unit(u)
                r0 = h2 * 64
                kt = 2 * ktp
                nc.tensor.matmul(
                    reg[:, slot0 * 512:slot0 * 512 + w],
                    kt_sb[j][r0:r0 + 64, kt * 128:(kt + 1) * 128],
                    qt_sb[j][r0:r0 + 64, qc4 * 512:qc4 * 512 + w],
                    start=True, stop=True,
                )

            def emit_av(u, ot, ptt, sb, first, last):
                # full-row AV (used for the small slot and at boundaries)
                qc4, j, ktp, h2 = unit(u)
                h = 2 * j + h2
                for s in range(2):
                    kt = 2 * ktp + s
                    c0 = (sb + s) * 512
                    nc.tensor.matmul(
                        ot[0:65, :],
                        v_sb[kt][:, h * 65:h * 65 + 65],
                        ptt[:, c0:c0 + 512],
                        start=(first and s == 0),
                        stop=(last and s == 1),
                    )

            def emit_av_pair(uA, uB, ptt):
                # split both units' AV into 64-row halves, interleaved so
                # adjacent matmuls alternate BOTH partition half and PSUM
                # bank (otA vs otB) -- concurrent same-bank accumulation
                # faults the PE (found the hard way)
                _, jA, ktpA, h2A = unit(uA)
                _, jB, ktpB, h2B = unit(uB)
                otA, otB = state["ot"][h2A], state["ot"][h2B]
                hA, hB = 2 * jA + h2A, 2 * jB + h2B
                for s in range(2):
                    ktA, ktB = 2 * ktpA + s, 2 * ktpB + s
                    for step in range(2):
                        hhA, hhB = step, 1 - step
                        cA, cB = s * 512, (2 + s) * 512
                        nc.tensor.matmul(
                            otA[0:65, :],
                            v_sb[ktA][hhA * 64:(hhA + 1) * 64, hA * 65:hA * 65 + 65],
                            ptt[hhA * 64:(hhA + 1) * 64, cA:cA + 512],
                            start=(ktpA == 0 and s == 0 and step == 0),
                            stop=(ktpA == 7 and s == 1 and step == 1),
                        )
                        nc.tensor.matmul(
                            otB[0:65, :],
                            v_sb[ktB][hhB * 64:(hhB + 1) * 64, hB * 65:hB * 65 + 65],
                            ptt[hhB * 64:(hhB + 1) * 64, cB:cB + 512],
                            start=(ktpB == 0 and s == 0 and step == 0),
                            stop=(ktpB == 7 and s == 1 and step == 1),
                        )

            def emit_norm(qc4, j, h2, ot):
                # otn[d, q] = ot[d, q] / ot[64, q] for this (block, head)
                drow = nrm.tile([1, 512], F32, tag="drow", name=f"dr{qc4}{j}{h2}")
                nc.vector.tensor_copy(drow[:], ot[64:65, :])
                otr = nrm.tile([HD, 512], BF16, tag="otr", name=f"otr{qc4}{j}{h2}")
                nc.vector.tensor_copy(otr[:], ot[0:HD, :])
                r32 = nrm.tile([1, 512], F32, tag="r32", name=f"r32{qc4}{j}{h2}")
                nc.vector.reciprocal_approx_fast(r32[:], drow[:])
                rb = nrm.tile([1, 512], BF16, tag="rb", name=f"rbn{qc4}{j}{h2}")
                nc.vector.tensor_copy(rb[:], r32[:])
                sc = nrm.tile([HD, 512], BF16, tag="sc", name=f"sc{qc4}{j}{h2}")
                nc.gpsimd.partition_broadcast(sc[:], rb[:])
                nc.vector.tensor_mul(
                    otn_sb[j][h2 * 64:h2 * 64 + 64,
                              qc4 * 512:qc4 * 512 + 512],
                    otr[:], sc[:])

            def emit_qp2(hp, pc):
                # Q projection for cols 1024:2048 (qc4 2,3): one [128,512]
                # column block on a boundary-window bank; evac on DVE only
                # (ACT is the stream bottleneck)
                acc = ot_tile(hp % 2, f"qp2_{hp}{pc}")
                for d in range(DCH):
                    if pc == 1:
                        xsrc = xq_sb[(d, 0)][:, 512:1024]
                    else:
                        xsrc = xq_sb[(d, 1)][:, (pc - 2) * 512:(pc - 1) * 512]
                    nc.tensor.matmul(
                        acc[:],
                        wq_sb[d][:, hp * 128:(hp + 1) * 128],
                        xsrc,
                        start=(d == 0), stop=(d == DCH - 1),
                    )
                nc.vector.tensor_scalar_add(
                    qt_sb[hp][:, pc * 512:(pc + 1) * 512], acc[:], bq_sb[hp])

            def emit_ytile(yt, tail=False):
                ysb = yp.tile([128, D], BF16, tag="y", name=f"ysb{yt}")
                for dc in range(2):
                    op = ot_tile(dc, f"op{yt}{dc}")
                    for g in range(2):
                        nc.tensor.matmul(
                            op[:],
                            otn_sb[g][:, yt * 128:(yt + 1) * 128],
                            wo_sb[g][:, dc * 512:(dc + 1) * 512],
                            start=(g == 0), stop=(g == 1),
                        )
                    if tail and dc == 1:
                        nc.scalar.copy(ysb[:, dc * 512:(dc + 1) * 512], op[:])
                    else:
                        nc.vector.tensor_copy(ysb[:, dc * 512:(dc + 1) * 512], op[:])
                    if tail:
                        nc.sync.dma_start(
                            y[yt * 128:(yt + 1) * 128, dc * 512:(dc + 1) * 512],
                            ysb[:, dc * 512:(dc + 1) * 512])
                if not tail:
                    nc.sync.dma_start(y[yt * 128:(yt + 1) * 128, :], ysb[:])

            # lump schedule per boundary index (after block b ends)
            #   b = 1..7 at units 16,32,...,112
            lumps = {
                1: [("qp2", 0, 1), ("qp2", 1, 1)],
                2: [("qp2", 0, 2), ("qp2", 1, 2)],
                3: [("y", 0), ("y", 1), ("y", 2), ("y", 3)],
                4: [("qp2", 0, 3), ("qp2", 1, 3)],
                5: [("y", 4), ("y", 5), ("y", 6), ("y", 7)],
                6: [("y", 8), ("y", 9), ("y", 10), ("y", 11)],
            }

            # ---- prologue: scores(0), scores(1) -> big0; scores(2) -> small0
            big = big_region(0)
            small = small_region(0)
            emit_scores(0, big, 0)
            emit_scores(1, big, 2)
            emit_scores(2, small, 0)

            state = {"ot": [ot_tile(0, "otA0"), ot_tile(1, "otB0")], "blk": 0}

            def block_end(qc4, j):
                emit_norm(qc4, j, 0, state["ot"][0])
                emit_norm(qc4, j, 1, state["ot"][1])
                state["blk"] += 1
                blk = state["blk"]
                for lump in lumps.get(blk, []):
                    if lump[0] == "qp2":
                        emit_qp2(lump[1], lump[2])
                    else:
                        emit_ytile(lump[1])
                state["ot"] = [ot_tile(0, f"otA{blk}"), ot_tile(1, f"otB{blk}")]

            def do_av(u, ptt, sb):
                qc4, j, ktp, h2 = unit(u)
                emit_av(u, state["ot"][h2], ptt, sb,
                        first=(ktp == 0), last=(ktp == 7))
                if ktp == 7 and h2 == 1:
                    block_end(qc4, j)

            for p in range(43):  # 43 periods x 3 units cover 128 units
                u0 = 3 * p
                ptb = ptb_t[p % 5]
                pts = pts_t[p % 5]
                # exp-big over units u0, u0+1
                if WIDE_EXP:
                    nc.scalar.activation(ptb[:], big[:], EXPF, scale=1.0 / SCALE)
                else:
                    nc.scalar.activation(ptb[:, 0:1024], big[:, 0:1024], EXPF,
                                         scale=1.0 / SCALE)
                    nc.scalar.activation(ptb[:, 1024:2048], big[:, 1024:2048],
                                         EXPF, scale=1.0 / SCALE)
                # refill big with scores(u0+3), (u0+4) in the exp shadow
                nxt_big = big_region(p + 1)
                if u0 + 4 < NU:
                    # interleave the two units' score matmuls: adjacent
                    # matmuls alternate partition halves -> PE row-tile
                    # packing (2x throughput on 64-contraction matmuls)
                    emit_scores(u0 + 3, nxt_big, 0, only=0)
                    emit_scores(u0 + 4, nxt_big, 2, only=0)
                    emit_scores(u0 + 3, nxt_big, 0, only=1)
                    emit_scores(u0 + 4, nxt_big, 2, only=1)
                    emit_filler(u0 + 3, nxt_big, 0)
                    emit_filler(u0 + 4, nxt_big, 2)
                elif u0 + 3 < NU:
                    emit_scores(u0 + 3, nxt_big, 0)
                    emit_filler(u0 + 3, nxt_big, 0)
                # AV stays full-row: split-half AV matmuls overlap under
                # PE row tiling and concurrent PSUM accumulation faults the
                # device (emit_av_pair kept for reference; do not enable)
                do_av(u0, ptb, 0)
                do_av(u0 + 1, ptb, 2)

                # exp-small over unit u0+2
                u2 = u0 + 2
                if u2 < NU:
                    nc.scalar.activation(pts[:], small[:], EXPF,
                                         scale=1.0 / SCALE)
                nxt_small = small_region(p + 1)
                if u2 + 3 < NU:
                    emit_scores(u2 + 3, nxt_small, 0)
                    emit_filler(u2 + 3, nxt_small, 0)
                if u2 < NU:
                    do_av(u2, pts, 0)

                big = nxt_big
                small = nxt_small

            # ---- tail: out-projection of q rows 1536:2048 on parallel
            # banks (ring + ot tags are all free now) ----
            for i in range(4):
                svd = ps_m.tile([128, 1024], F32, tag="rs", name=f"warmt{i}")
                nc.tensor.matmul(svd[:, 0:512], wrm[:, 0:128], wrm[:],
                                 start=True, stop=True)
            tob = ps_m.tile([128, 2048], F32, tag="rb", name="tailb")
            tos = ps_m.tile([128, 1024], F32, tag="rs", name="tails")
            taccs = [tob[:, 0:512], tob[:, 512:1024], tob[:, 1024:1536],
                     tob[:, 1536:2048], tos[:, 0:512], tos[:, 512:1024],
                     ot_tile(0, "tailo0")[:], ot_tile(1, "tailo1")[:]]
            for i, yt in enumerate(range(12, 16)):
                ysb = yp.tile([128, D], BF16, tag="y", name=f"ysb{yt}")
                for dc in range(2):
                    op = taccs[i * 2 + dc]
                    for g in range(2):
                        nc.tensor.matmul(
                            op,
                            otn_sb[g][:, yt * 128:(yt + 1) * 128],
                            wo_sb[g][:, dc * 512:(dc + 1) * 512],
                            start=(g == 0), stop=(g == 1),
                        )
                    if dc == 1:
                        nc.scalar.copy(ysb[:, dc * 512:(dc + 1) * 512], op)
                    else:
                        nc.vector.tensor_copy(ysb[:, dc * 512:(dc + 1) * 512], op)
                    nc.sync.dma_start(
                        y[yt * 128:(yt + 1) * 128, dc * 512:(dc + 1) * 512],
                        ysb[:, dc * 512:(dc + 1) * 512])

    nc.compile()
    return nc


_NC_CACHE = None


def _get_nc():
    global _NC_CACHE
    if _NC_CACHE is None:
        _NC_CACHE = build_nc()
    return _NC_CACHE


def shard_inputs(query, key, value, Wq, bq, Wk, bk, Wv, bv, Wo, bo):
    """Build the 8 per-core input maps (host-side shard + transpose)."""
    import ml_dtypes
    f = np.float32
    bf = ml_dtypes.bfloat16
    in_maps = []
    for c in range(NCORES):
        b = c // 4
        g = c % 4
        hs = slice(g * DPC, (g + 1) * DPC)
        in_maps.append({
            "xq_t": np.ascontiguousarray(np.asarray(query[b], f).T.astype(bf)),
            "xk_t": np.ascontiguousarray(np.asarray(key[b], f).T.astype(bf)),
            "xv_t": np.ascontiguousarray(np.asarray(value[b], f).T.astype(bf)),
            "wq_t": np.ascontiguousarray(np.asarray(Wq[hs, :], f).T.astype(bf)),
            "wk_t": np.ascontiguousarray(np.asarray(Wk[hs, :], f).T.astype(bf)),
            "wv_t": np.ascontiguousarray(np.asarray(Wv[hs, :], f).T.astype(bf)),
            "wo_t": np.ascontiguousarray(np.asarray(Wo[:, hs], f).T.astype(bf)),
            "bq": np.asarray(bq[hs], f).reshape(DPC, 1).copy(),
            "bk": np.asarray(bk[hs], f).reshape(DPC, 1).copy(),
        })
    return in_maps


def kernel(query, key, value, Wq, bq, Wk, bk, Wv, bv, Wo, bo, **run_kwargs):
    nc = _get_nc()
    in_maps = shard_inputs(query, key, value, Wq, bq, Wk, bk, Wv, bv, Wo, bo)
    res = run_bass_kernel_spmd(nc, in_maps, core_ids=list(range(NCORES)),
                               **run_kwargs)
    out = np.zeros((B, S, D), np.float32)
    for c in range(NCORES):
        out[c // 4] += np.asarray(res.results[c]["y"], np.float32)
    # V bias folded here: attention rows sum to 1, so +bv passes through
    # attention unchanged and contributes bv @ Wo.T to every output row.
    bo_eff = np.asarray(bo, np.float32) + np.asarray(bv, np.float32) @ np.asarray(Wo, np.float32).T
    out += bo_eff
    if run_kwargs:
        kernel.last_result = res
    return out
